# revision 1
# baseline (speedup 1.0000x reference)
"""Trainium2 Bass kernel for KMGCN (2x GCNConv + global mean pool + FC), 8 cores.

Sharding: dst-nodes partitioned contiguously across 8 cores (6250 each).
Edge messages are pre-permuted on host (pure index gather) into per-core
sequential streams; the device does all arithmetic:
  - one-hot scatter matmuls (PSUM accumulation) for sym-normalized aggregation
  - dense matmuls for the W1/W2 transforms, ReLU+bias on ACT/DVE
  - matmul pooling with a per-core P matrix (1/cnt one-hot), AllReduce, FC.
Two launches: L1 produces the h2pre table (h1 @ W2); host permutes rows by
src index; L2 aggregates, pools, and applies the FC.
"""

import numpy as np
import concourse.bass as bass
import concourse.bacc as bacc
import concourse.tile as tile
import concourse.mybir as mybir
from concourse.bass_utils import run_bass_kernel_spmd

NCORES = 8
F32 = mybir.dt.float32
C_CALL = 32  # chunks per DMA call

_cache = {}
last_result = None
exec_wall = [0.0, 0.0]


def _plan(src, dst, n_nodes):
    """Static schedule: per-core chunked edge lists, padded so all cores share
    one program. Returns per-core edge arrays + chunk->tile map."""
    npc = n_nodes // NCORES
    deg = np.bincount(dst, minlength=n_nodes).astype(np.float32) + 1.0
    dinv = 1.0 / np.sqrt(deg)
    # edges + self loops
    a_src = np.concatenate([src, np.arange(n_nodes, dtype=src.dtype)])
    a_dst = np.concatenate([dst, np.arange(n_nodes, dtype=src.dtype)])
    a_w = (dinv[a_src] * dinv[a_dst]).astype(np.float32)

    ntile = (npc + 127) // 128
    per_core = []
    counts = np.zeros((NCORES, ntile), np.int64)
    for c in range(NCORES):
        m = (a_dst >= c * npc) & (a_dst < (c + 1) * npc)
        es, ed, ew = a_src[m], a_dst[m] - c * npc, a_w[m]
        order = np.argsort(ed, kind="stable")
        es, ed, ew = es[order], ed[order], ew[order]
        per_core.append((es, ed, ew))
        tl = ed // 128
        cnt = np.bincount(tl, minlength=ntile)
        counts[c] = cnt
    cpt = np.maximum(1, (np.ceil(counts.max(0) / 128.0)).astype(np.int64))
    nch = int(cpt.sum())
    ncalls = (nch + C_CALL - 1) // C_CALL
    nchp = ncalls * C_CALL

    cores = []
    for c in range(NCORES):
        es, ed, ew = per_core[c]
        gs = np.zeros(nchp * 128, np.int64)
        sd = np.zeros(nchp * 128, np.float32)
        sw = np.zeros(nchp * 128, np.float32)
        pos = 0
        start = 0
        tl = ed // 128
        bounds = np.searchsorted(tl, np.arange(ntile + 1))
        for t in range(ntile):
            lo, hi = bounds[t], bounds[t + 1]
            n = hi - lo
            gs[pos : pos + n] = es[lo:hi]
            sd[pos : pos + n] = (ed[lo:hi] - t * 128).astype(np.float32)
            sw[pos : pos + n] = ew[lo:hi]
            pos += int(cpt[t]) * 128
        cores.append((gs, sd, sw))
    return dict(npc=npc, ntile=ntile, cpt=cpt, nch=nch, ncalls=ncalls, nchp=nchp,
                cores=cores, dinv=dinv)


def _pack_calls(vals, ncalls, width):
    """[nchp*128] -> [ncalls, 128, C_CALL*width] with edge (call k, chunk c,
    lane p) at [k, p, c*width:(c+1)*width]."""
    x = vals.reshape(ncalls, C_CALL, 128, width)      # [k, c, p, w]
    return np.ascontiguousarray(x.transpose(0, 2, 1, 3)).reshape(
        ncalls, 128, C_CALL * width)


def _build_l1(meta, in_dim, hid):
    ntile, cpt, ncalls = meta["ntile"], meta["cpt"], meta["ncalls"]
    npad = ntile * 128
    npc = meta["npc"]
    nc = bacc.Bacc("TRN2", target_bir_lowering=False, debug=False,
                   num_devices=NCORES)
    t_xg = nc.dram_tensor("xg", [ncalls, 128, C_CALL * in_dim], F32, kind="ExternalInput")
    t_sd = nc.dram_tensor("sd", [ncalls, 128, C_CALL], F32, kind="ExternalInput")
    t_sw = nc.dram_tensor("sw", [ncalls, 128, C_CALL], F32, kind="ExternalInput")
    t_w1 = nc.dram_tensor("w1", [in_dim, hid], F32, kind="ExternalInput")
    t_b1 = nc.dram_tensor("b1", [128, 2], F32, kind="ExternalInput")
    t_w2 = nc.dram_tensor("w2", [hid, hid // 2], F32, kind="ExternalInput")
    t_iota = nc.dram_tensor("iota", [128, 128], F32, kind="ExternalInput")
    t_eye = nc.dram_tensor("eye", [128, 128], F32, kind="ExternalInput")
    t_out = nc.dram_tensor("h2pre", [npad, hid // 2], F32, kind="ExternalOutput")

    nh = hid // 128          # 2 halves of hid (256)
    oh = hid // 2            # 128 out dim of layer 2 pre
    with tile.TileContext(nc) as tc:
        with (
            tc.tile_pool(name="consts", bufs=1) as cp,
            tc.tile_pool(name="gp", bufs=3) as gp,
            tc.tile_pool(name="sp", bufs=3) as sp,
            tc.tile_pool(name="persist", bufs=1) as pp,
            tc.tile_pool(name="stage", bufs=3) as stp,
            tc.tile_pool(name="ps_agg", bufs=2, space="PSUM") as ps_agg,
            tc.tile_pool(name="ps_big", bufs=2, space="PSUM") as ps_big,
            tc.tile_pool(name="ps_tr", bufs=2, space="PSUM") as ps_tr,
        ):
            iota = cp.tile([128, 128], F32)
            eye = cp.tile([128, 128], F32)
            w1 = cp.tile([in_dim, hid], F32)
            b1 = cp.tile([128, 2], F32)
            w2a = cp.tile([128, oh], F32)
            w2b = cp.tile([128, oh], F32)
            nc.sync.dma_start(out=iota[:, :], in_=t_iota[:, :])
            nc.sync.dma_start(out=eye[:, :], in_=t_eye[:, :])
            nc.sync.dma_start(out=w1[:, :], in_=t_w1[:, :])
            nc.sync.dma_start(out=b1[:, :], in_=t_b1[:, :])
            nc.sync.dma_start(out=w2a[:, :], in_=t_w2[0:128, :])
            nc.sync.dma_start(out=w2b[:, :], in_=t_w2[128:256, :])

            agg1 = pp.tile([128, ntile * 128], F32)   # agg1^T, feat-major
            h1a = pp.tile([128, ntile * 128], F32)    # h1^T half 0
            h1b = pp.tile([128, ntile * 128], F32)    # h1^T half 1

            # scatter phase: chunks stream call by call
            ch = 0
            call_t = None
            for t in range(ntile):
                pt = ps_agg.tile([128, 128], F32, name=f"agg_ps{t % 2}", tag="aggps")
                for j in range(int(cpt[t])):
                    k, cc = ch // C_CALL, ch % C_CALL
                    if cc == 0:
                        call_t = gp.tile([128, C_CALL * in_dim], F32, tag="g")
                        nc.sync.dma_start(out=call_t[:, :], in_=t_xg[k, :, :])
                        sd_t = sp.tile([128, C_CALL], F32, tag="sd")
                        sw_t = sp.tile([128, C_CALL], F32, tag="sw")
                        nc.sync.dma_start(out=sd_t[:, :], in_=t_sd[k, :, :])
                        nc.sync.dma_start(out=sw_t[:, :], in_=t_sw[k, :, :])
                    s_t = sp.tile([128, 128], F32, tag="s", bufs=4)
                    nc.vector.tensor_scalar(
                        out=s_t[:, :], in0=iota[:, :],
                        scalar1=sd_t[:, cc : cc + 1], scalar2=sw_t[:, cc : cc + 1],
                        op0=mybir.AluOpType.is_equal, op1=mybir.AluOpType.mult)
                    nc.tensor.matmul(
                        pt[:, :], lhsT=call_t[:, cc * in_dim : (cc + 1) * in_dim],
                        rhs=s_t[:, :], start=(j == 0), stop=(j == int(cpt[t]) - 1))
                    ch += 1
                nc.vector.tensor_copy(agg1[:, t * 128 : (t + 1) * 128], pt[:, :])

            # transform: h1^T = relu(W1^T agg1 + b1), in column groups of 512
            for g0 in range(0, ntile * 128, 512):
                g1 = min(g0 + 512, ntile * 128)
                for h, (dstb, w1s) in enumerate(
                    [(h1a, w1[:, 0:128]), (h1b, w1[:, 128:256])][:nh]
                ):
                    pb = ps_big.tile([128, 512], F32, tag="big")
                    nc.tensor.matmul(pb[:, : g1 - g0], lhsT=w1s, rhs=agg1[:, g0:g1],
                                     start=True, stop=True)
                    nc.scalar.activation(
                        out=dstb[:, g0:g1], in_=pb[:, : g1 - g0],
                        func=mybir.ActivationFunctionType.Relu,
                        bias=b1[:, h : h + 1], scale=1.0)

            # h2pre^T = W2^T h1 then transpose to row-major and store
            for g0 in range(0, ntile * 128, 512):
                g1 = min(g0 + 512, ntile * 128)
                pb = ps_big.tile([128, 512], F32, tag="big")
                nc.tensor.matmul(pb[:, : g1 - g0], lhsT=w2a[:, :], rhs=h1a[:, g0:g1],
                                 start=True, stop=False)
                nc.tensor.matmul(pb[:, : g1 - g0], lhsT=w2b[:, :], rhs=h1b[:, g0:g1],
                                 start=False, stop=True)
                hp = stp.tile([128, 512], F32, tag="hp")
                nc.vector.tensor_copy(hp[:, : g1 - g0], pb[:, : g1 - g0])
                for b0 in range(g0, g1, 128):
                    ptr = ps_tr.tile([128, 128], F32, tag="tr")
                    nc.tensor.transpose(ptr[:, :], hp[:, b0 - g0 : b0 - g0 + 128],
                                        eye[:, :])
                    ro = stp.tile([128, 128], F32, tag="ro")
                    nc.vector.tensor_copy(ro[:, :], ptr[:, :])
                    nc.sync.dma_start(out=t_out[b0 : b0 + 128, :], in_=ro[:, :])
    nc.compile()
    return nc


def _build_l2(meta, oh, n_graphs):
    ntile, cpt, ncalls = meta["ntile"], meta["cpt"], meta["ncalls"]
    npad = ntile * 128
    nc = bacc.Bacc("TRN2", target_bir_lowering=False, debug=False,
                   num_devices=NCORES)
    t_hg = nc.dram_tensor("hg", [ncalls, 128, C_CALL * oh], F32, kind="ExternalInput")
    t_sd = nc.dram_tensor("sd", [ncalls, 128, C_CALL], F32, kind="ExternalInput")
    t_sw = nc.dram_tensor("sw", [ncalls, 128, C_CALL], F32, kind="ExternalInput")
    t_b2r = nc.dram_tensor("b2r", [128, oh], F32, kind="ExternalInput")
    t_pm = nc.dram_tensor("pm", [npad, n_graphs], F32, kind="ExternalInput")
    t_wfc = nc.dram_tensor("wfc", [oh, 8], F32, kind="ExternalInput")
    t_bfc = nc.dram_tensor("bfc", [n_graphs, 8], F32, kind="ExternalInput")
    t_iota = nc.dram_tensor("iota", [128, 128], F32, kind="ExternalInput")
    t_out = nc.dram_tensor("out", [n_graphs, 8], F32, kind="ExternalOutput")

    with tile.TileContext(nc) as tc:
        with (
            tc.tile_pool(name="consts", bufs=1) as cp,
            tc.tile_pool(name="gp", bufs=3) as gp,
            tc.tile_pool(name="sp", bufs=3) as sp,
            tc.tile_pool(name="stage", bufs=4) as stp,
            tc.tile_pool(name="dram", bufs=1, space="DRAM") as dp,
            tc.tile_pool(name="ps_agg", bufs=4, space="PSUM") as ps_agg,
            tc.tile_pool(name="ps_pool", bufs=1, space="PSUM") as ps_pool,
            tc.tile_pool(name="ps_fc", bufs=1, space="PSUM") as ps_fc,
        ):
            iota = cp.tile([128, 128], F32)
            b2r = cp.tile([128, oh], F32)
            wfc = cp.tile([oh, 8], F32)
            bfc = cp.tile([n_graphs, 8], F32)
            nc.sync.dma_start(out=iota[:, :], in_=t_iota[:, :])
            nc.sync.dma_start(out=b2r[:, :], in_=t_b2r[:, :])
            nc.sync.dma_start(out=wfc[:, :], in_=t_wfc[:, :])
            nc.sync.dma_start(out=bfc[:, :], in_=t_bfc[:, :])

            ar_in = dp.tile([128, n_graphs], F32)
            ar_out = dp.tile([128, n_graphs], F32, addr_space="Shared")

            ppool = ps_pool.tile([128, n_graphs], F32)
            ch = 0
            call_t = None
            for t in range(ntile):
                pt = ps_agg.tile([128, 128], F32, tag="aggps")
                for j in range(int(cpt[t])):
                    k, cc = ch // C_CALL, ch % C_CALL
                    if cc == 0:
                        call_t = gp.tile([128, C_CALL * oh], F32, tag="g")
                        nc.sync.dma_start(out=call_t[:, :], in_=t_hg[k, :, :])
                        sd_t = sp.tile([128, C_CALL], F32, tag="sd")
                        sw_t = sp.tile([128, C_CALL], F32, tag="sw")
                        nc.sync.dma_start(out=sd_t[:, :], in_=t_sd[k, :, :])
                        nc.sync.dma_start(out=sw_t[:, :], in_=t_sw[k, :, :])
                    s_t = sp.tile([128, 128], F32, tag="s", bufs=4)
                    nc.vector.tensor_scalar(
                        out=s_t[:, :], in0=iota[:, :],
                        scalar1=sd_t[:, cc : cc + 1], scalar2=sw_t[:, cc : cc + 1],
                        op0=mybir.AluOpType.is_equal, op1=mybir.AluOpType.mult)
                    # node-major: out[nodes, feat] = S^T @ G
                    nc.tensor.matmul(
                        pt[:, :], lhsT=s_t[:, :],
                        rhs=call_t[:, cc * oh : (cc + 1) * oh],
                        start=(j == 0), stop=(j == int(cpt[t]) - 1))
                    ch += 1
                h2 = stp.tile([128, oh], F32, tag="h2")
                nc.vector.tensor_tensor(out=h2[:, :], in0=pt[:, :], in1=b2r[:, :],
                                        op=mybir.AluOpType.add)
                nc.vector.tensor_scalar(
                    out=h2[:, :], in0=h2[:, :], scalar1=0.0, scalar2=None,
                    op0=mybir.AluOpType.max)
                pm_t = sp.tile([128, n_graphs], F32, tag="pm")
                nc.sync.dma_start(out=pm_t[:, :], in_=t_pm[t * 128 : (t + 1) * 128, :])
                nc.tensor.matmul(ppool[:, :], lhsT=h2[:, :], rhs=pm_t[:, :],
                                 start=(t == 0), stop=(t == ntile - 1))

            pooled = stp.tile([128, n_graphs], F32, tag="pooled")
            nc.vector.tensor_copy(pooled[:, :], ppool[:, :])
            nc.sync.dma_start(out=ar_in[:, :], in_=pooled[:, :])
            nc.gpsimd.collective_compute(
                "AllReduce", mybir.AluOpType.add,
                replica_groups=[list(range(NCORES))],
                ins=[ar_in[:, :].opt()], outs=[ar_out[:, :].opt()])
            pfull = stp.tile([128, n_graphs], F32, tag="pfull")
            nc.sync.dma_start(out=pfull[:, :], in_=ar_out[:, :])
            pfc = ps_fc.tile([n_graphs, 8], F32)
            nc.tensor.matmul(pfc[:, :], lhsT=pfull[:, :], rhs=wfc[:, :],
                             start=True, stop=True)
            osb = stp.tile([n_graphs, 8], F32, tag="osb")
            nc.vector.tensor_tensor(out=osb[:, :], in0=pfc[:, :], in1=bfc[:, :],
                                    op=mybir.AluOpType.add)
            nc.sync.dma_start(out=t_out[:, :], in_=osb[:, :])
    nc.compile()
    return nc


def kernel(x, src, dst, batch, W1, b1, W2, b2, Wfc, bfc):
    global last_result
    x = np.asarray(x, np.float32)
    src = np.asarray(src, np.int64)
    dst = np.asarray(dst, np.int64)
    batch = np.asarray(batch, np.int64)
    W1, b1v, W2, b2v, Wfc, bfcv = (np.asarray(a, np.float32)
                                   for a in (W1, b1, W2, b2, Wfc, bfc))
    n, in_dim = x.shape
    hid = W1.shape[1]
    oh = W2.shape[1]
    ng = 64
    odim = Wfc.shape[1]

    meta = _plan(src, dst, n)
    npc, ntile, ncalls, nchp = meta["npc"], meta["ntile"], meta["ncalls"], meta["nchp"]
    npad = ntile * 128

    key = (n, in_dim, hid, oh, tuple(meta["cpt"]))
    if key not in _cache:
        _cache[key] = (_build_l1(meta, in_dim, hid), _build_l2(meta, oh, ng))
    nc1, nc2 = _cache[key]

    iota = np.tile(np.arange(128, dtype=np.float32), (128, 1))
    eye = np.eye(128, dtype=np.float32)

    # ---- launch 1: host-gather x rows per core ----
    in1 = []
    for c in range(NCORES):
        gs, sd, sw = meta["cores"][c]
        xg = _pack_calls(x[gs], ncalls, in_dim)
        in1.append({
            "xg": xg,
            "sd": _pack_calls(sd, ncalls, 1).reshape(ncalls, 128, C_CALL),
            "sw": _pack_calls(sw, ncalls, 1).reshape(ncalls, 128, C_CALL),
            "w1": W1, "b1": np.ascontiguousarray(b1v.reshape(2, 128).T), "w2": W2,
            "iota": iota, "eye": eye,
        })
    import time as _t
    _s = _t.time()
    r1 = run_bass_kernel_spmd(nc1, in1, core_ids=list(range(NCORES)))
    exec_wall[0] = _t.time() - _s
    h2pre = np.concatenate([r1.results[c]["h2pre"][:npc] for c in range(NCORES)], 0)

    # ---- launch 2: host-gather h2pre rows, aggregate, pool, FC ----
    cnt = np.bincount(batch, minlength=ng).astype(np.float32)
    cnt = np.maximum(cnt, 1.0)
    b2r = np.tile(b2v.reshape(1, oh), (128, 1)).astype(np.float32)
    wfc8 = np.zeros((oh, 8), np.float32)
    wfc8[:, :odim] = Wfc
    bfc8 = np.zeros((ng, 8), np.float32)
    bfc8[:, :odim] = bfcv.reshape(1, odim)
    in2 = []
    for c in range(NCORES):
        gs, sd, sw = meta["cores"][c]
        hg = _pack_calls(h2pre[gs], ncalls, oh)
        pm = np.zeros((npad, ng), np.float32)
        nl = np.arange(npc) + c * npc
        pm[np.arange(npc), batch[nl]] = 1.0 / cnt[batch[nl]]
        in2.append({
            "hg": hg,
            "sd": in1[c]["sd"], "sw": in1[c]["sw"],
            "b2r": b2r, "pm": pm, "wfc": wfc8, "bfc": bfc8, "iota": iota,
        })
    _s = _t.time()
    r2 = run_bass_kernel_spmd(nc2, in2, core_ids=list(range(NCORES)))
    exec_wall[1] = _t.time() - _s
    last_result = (r1, r2)
    return np.asarray(r2.results[0]["out"][:, :odim], np.float32)



# revision 5
# speedup vs baseline: 9.9840x; 9.9840x over previous
"""Trainium2 Bass kernel for KMGCN (2x GCNConv + global mean pool + FC), 8 cores.

Single launch, on-device edge gather:
  - dst-nodes partitioned contiguously across 8 cores (6250 each); host ships
    only the x shard plus per-edge metadata (src index / dst slot / weight),
    ~6MB per core instead of pre-gathered features.
  - x shards are AllGathered on device into a full [50000,128] HBM table;
    edge messages gather from it with indirect DMA (128 rows per call).
  - sym-normalized aggregation via one-hot scatter matmuls (PSUM
    accumulation), dense W1/W2 transforms on PE, ReLU+bias on ACT/DVE.
  - the layer-2 table (h1 @ W2, node-major) is built on device (TensorE
    transpose) and AllGathered; layer-2 aggregates node-major, pools via a
    per-graph one-hot matmul, AllReduces, and applies the FC.
"""

import numpy as np
import concourse.bass as bass
import concourse.bacc as bacc
import concourse.tile as tile
import concourse.mybir as mybir
from concourse.bass_utils import run_bass_kernel_spmd

NCORES = 8
F32 = mybir.dt.float32
I32 = mybir.dt.int32
CB = 32  # chunks per metadata DMA block

_cache = {}
last_result = None
exec_wall = [0.0]


def _plan(src, dst, n_nodes):
    """Static schedule: per-core chunked edge lists, padded so all cores share
    one program. Edge (call k, chunk c, lane p) lives at [k, p, c]."""
    npc = n_nodes // NCORES
    deg = np.bincount(dst, minlength=n_nodes).astype(np.float32) + 1.0
    dinv = 1.0 / np.sqrt(deg)
    a_src = np.concatenate([src, np.arange(n_nodes, dtype=src.dtype)])
    a_dst = np.concatenate([dst, np.arange(n_nodes, dtype=src.dtype)])
    a_w = (dinv[a_src] * dinv[a_dst]).astype(np.float32)

    ntile = (npc + 127) // 128
    per_core = []
    counts = np.zeros((NCORES, ntile), np.int64)
    for c in range(NCORES):
        m = (a_dst >= c * npc) & (a_dst < (c + 1) * npc)
        es, ed, ew = a_src[m], a_dst[m] - c * npc, a_w[m]
        order = np.argsort(ed, kind="stable")
        es, ed, ew = es[order], ed[order], ew[order]
        per_core.append((es, ed, ew))
        counts[c] = np.bincount(ed // 128, minlength=ntile)
    cpt = np.maximum(1, (np.ceil(counts.max(0) / 128.0)).astype(np.int64))
    nch = int(cpt.sum())
    ncalls = (nch + CB - 1) // CB
    nchp = ncalls * CB

    cores = []
    for c in range(NCORES):
        es, ed, ew = per_core[c]
        gs = np.zeros(nchp * 128, np.int32)
        sd = np.zeros(nchp * 128, np.float32)
        sw = np.zeros(nchp * 128, np.float32)
        tl = ed // 128
        bounds = np.searchsorted(tl, np.arange(ntile + 1))
        pos = 0
        for t in range(ntile):
            lo, hi = bounds[t], bounds[t + 1]
            n = hi - lo
            gs[pos : pos + n] = es[lo:hi]
            sd[pos : pos + n] = (ed[lo:hi] - t * 128).astype(np.float32)
            sw[pos : pos + n] = ew[lo:hi]
            pos += int(cpt[t]) * 128
        cores.append((gs, sd, sw))
    return dict(npc=npc, ntile=ntile, cpt=cpt, nch=nch, ncalls=ncalls, nchp=nchp,
                cores=cores)


def _pack_calls(vals, ncalls):
    """[nchp*128] -> [ncalls, 128, CB] with edge (k, chunk c, lane p) at
    [k, p, c]."""
    return np.ascontiguousarray(
        vals.reshape(ncalls, CB, 128).transpose(0, 2, 1))


def _build(meta, n_nodes, in_dim, hid, oh, n_graphs):
    ntile, cpt, ncalls = meta["ntile"], meta["cpt"], meta["ncalls"]
    npc = meta["npc"]
    npad = ntile * 128
    nc = bacc.Bacc("TRN2", target_bir_lowering=False, debug=False,
                   num_devices=NCORES)
    t_xs = nc.dram_tensor("xs", [npc, in_dim], F32, kind="ExternalInput")
    t_gi = nc.dram_tensor("gi", [ncalls, 128, CB], I32, kind="ExternalInput")
    t_sd = nc.dram_tensor("sd", [ncalls, 128, CB], F32, kind="ExternalInput")
    t_sw = nc.dram_tensor("sw", [ncalls, 128, CB], F32, kind="ExternalInput")
    t_w1 = nc.dram_tensor("w1", [in_dim, hid], F32, kind="ExternalInput")
    t_b1 = nc.dram_tensor("b1", [128, hid // 128], F32, kind="ExternalInput")
    t_w2 = nc.dram_tensor("w2", [hid, oh], F32, kind="ExternalInput")
    t_b2r = nc.dram_tensor("b2r", [128, oh], F32, kind="ExternalInput")
    t_pms = nc.dram_tensor("pms", [128, ntile * 2], F32, kind="ExternalInput")
    t_wfc = nc.dram_tensor("wfc", [oh, 8], F32, kind="ExternalInput")
    t_bfc = nc.dram_tensor("bfc", [n_graphs, 8], F32, kind="ExternalInput")
    t_iota = nc.dram_tensor("iota", [128, 128], F32, kind="ExternalInput")
    t_eye = nc.dram_tensor("eye", [128, 128], F32, kind="ExternalInput")
    t_out = nc.dram_tensor("out", [n_graphs, 8], F32, kind="ExternalOutput")

    nh = hid // 128
    with tile.TileContext(nc) as tc:
        with (
            tc.tile_pool(name="xfull", bufs=1, space="DRAM") as xfp,
            tc.tile_pool(name="hfull", bufs=1, space="DRAM") as hfp,
            tc.tile_pool(name="ccs", bufs=1, space="DRAM") as ccp,
            tc.tile_pool(name="consts", bufs=1) as cp,
            tc.tile_pool(name="meta", bufs=3) as mp,
            tc.tile_pool(name="gath", bufs=4) as gp,
            tc.tile_pool(name="sbs", bufs=4) as sp,
            tc.tile_pool(name="persist", bufs=1) as pp,
            tc.tile_pool(name="stage", bufs=3) as stp,
            tc.tile_pool(name="ps_agg", bufs=2, space="PSUM") as ps_agg,
            tc.tile_pool(name="ps_big", bufs=2, space="PSUM") as ps_big,
            tc.tile_pool(name="ps_tr", bufs=2, space="PSUM") as ps_tr,
            tc.tile_pool(name="ps_pool", bufs=1, space="PSUM") as ps_pool,
            tc.tile_pool(name="ps_fc", bufs=1, space="PSUM") as ps_fc,
        ):
            # ---- constants ----
            iota = cp.tile([128, 128], F32)
            eye = cp.tile([128, 128], F32)
            w1 = cp.tile([in_dim, hid], F32)
            b1 = cp.tile([128, nh], F32)
            w2a = cp.tile([128, oh], F32)
            w2b = cp.tile([128, oh], F32)
            b2r = cp.tile([128, oh], F32)
            wfc = cp.tile([oh, 8], F32)
            bfc = cp.tile([n_graphs, 8], F32)
            pms = cp.tile([128, ntile * 2], F32)
            nc.sync.dma_start(out=iota[:, :], in_=t_iota[:, :])
            nc.sync.dma_start(out=eye[:, :], in_=t_eye[:, :])
            nc.sync.dma_start(out=w1[:, :], in_=t_w1[:, :])
            nc.sync.dma_start(out=b1[:, :], in_=t_b1[:, :])
            nc.sync.dma_start(out=w2a[:, :], in_=t_w2[0:128, :])
            nc.sync.dma_start(out=w2b[:, :], in_=t_w2[128:256, :])
            nc.sync.dma_start(out=b2r[:, :], in_=t_b2r[:, :])
            nc.sync.dma_start(out=wfc[:, :], in_=t_wfc[:, :])
            nc.sync.dma_start(out=bfc[:, :], in_=t_bfc[:, :])
            nc.sync.dma_start(out=pms[:, :], in_=t_pms[:, :])

            # ---- AllGather x shards into the full gather table ----
            cc_x = ccp.tile([npc, in_dim], F32)
            cc_h = ccp.tile([npc, oh], F32)
            x_full = xfp.tile([n_nodes, in_dim], F32, addr_space="Shared")
            h_full = hfp.tile([n_nodes, oh], F32, addr_space="Shared")
            nc.sync.dma_start(out=cc_x[:, :], in_=t_xs[:, :])
            nc.gpsimd.collective_compute(
                "AllGather", mybir.AluOpType.bypass,
                replica_groups=[list(range(NCORES))],
                ins=[cc_x[:, :].opt()], outs=[x_full[:, :].opt()])

            agg1 = pp.tile([128, npad], F32)   # agg1^T (feature-major)
            h1a = pp.tile([128, npad], F32)    # h1^T half 0
            h1b = pp.tile([128, npad], F32)    # h1^T half 1

            # ---- L1 scatter: agg1^T[:, tile] = sum_e w_e x[src_e]^T ----
            ch = 0
            gi_t = sd_t = sw_t = None
            for t in range(ntile):
                pt = ps_agg.tile([128, 128], F32, tag="aggps")
                for j in range(int(cpt[t])):
                    k, cc = ch // CB, ch % CB
                    if cc == 0:
                        gi_t = mp.tile([128, CB], I32, tag="gi")
                        sd_t = mp.tile([128, CB], F32, tag="sd")
                        sw_t = mp.tile([128, CB], F32, tag="sw")
                        nc.sync.dma_start(out=gi_t[:, :], in_=t_gi[k, :, :])
                        nc.sync.dma_start(out=sd_t[:, :], in_=t_sd[k, :, :])
                        nc.sync.dma_start(out=sw_t[:, :], in_=t_sw[k, :, :])
                    g_t = gp.tile([128, in_dim], F32, tag="g")
                    nc.gpsimd.indirect_dma_start(
                        out=g_t[:, :], out_offset=None, in_=x_full[:, :],
                        in_offset=bass.IndirectOffsetOnAxis(
                            ap=gi_t[:, cc : cc + 1], axis=0))
                    s_t = sp.tile([128, 128], F32, tag="s")
                    nc.vector.tensor_scalar(
                        out=s_t[:, :], in0=iota[:, :],
                        scalar1=sd_t[:, cc : cc + 1], scalar2=sw_t[:, cc : cc + 1],
                        op0=mybir.AluOpType.is_equal, op1=mybir.AluOpType.mult)
                    nc.tensor.matmul(pt[:, :], lhsT=g_t[:, :], rhs=s_t[:, :],
                                     start=(j == 0), stop=(j == int(cpt[t]) - 1))
                    ch += 1
                nc.vector.tensor_copy(agg1[:, t * 128 : (t + 1) * 128], pt[:, :])

            # ---- L1 transform: h1^T = relu(W1^T agg1 + b1) ----
            for g0 in range(0, npad, 512):
                g1 = min(g0 + 512, npad)
                for h, dstb in enumerate([h1a, h1b][:nh]):
                    pb = ps_big.tile([128, 512], F32, tag="big")
                    nc.tensor.matmul(pb[:, : g1 - g0],
                                     lhsT=w1[:, h * 128 : (h + 1) * 128],
                                     rhs=agg1[:, g0:g1], start=True, stop=True)
                    nc.scalar.activation(
                        out=dstb[:, g0:g1], in_=pb[:, : g1 - g0],
                        func=mybir.ActivationFunctionType.Relu,
                        bias=b1[:, h : h + 1], scale=1.0)

            # ---- h2pre^T = W2^T h1, transpose to node-major, AllGather ----
            for g0 in range(0, npad, 512):
                g1 = min(g0 + 512, npad)
                pb = ps_big.tile([128, 512], F32, tag="big")
                nc.tensor.matmul(pb[:, : g1 - g0], lhsT=w2a[:, :], rhs=h1a[:, g0:g1],
                                 start=True, stop=False)
                nc.tensor.matmul(pb[:, : g1 - g0], lhsT=w2b[:, :], rhs=h1b[:, g0:g1],
                                 start=False, stop=True)
                hp = stp.tile([128, 512], F32, tag="hp")
                nc.vector.tensor_copy(hp[:, : g1 - g0], pb[:, : g1 - g0])
                for b0 in range(g0, g1, 128):
                    ptr = ps_tr.tile([128, 128], F32, tag="tr")
                    nc.tensor.transpose(ptr[:, :], hp[:, b0 - g0 : b0 - g0 + 128],
                                        eye[:, :])
                    ro = stp.tile([128, 128], F32, tag="ro")
                    nc.vector.tensor_copy(ro[:, :], ptr[:, :])
                    nr = min(128, npc - b0)
                    if nr > 0:
                        nc.sync.dma_start(out=cc_h[b0 : b0 + nr, :],
                                          in_=ro[:nr, :])
            nc.gpsimd.collective_compute(
                "AllGather", mybir.AluOpType.bypass,
                replica_groups=[list(range(NCORES))],
                ins=[cc_h[:, :].opt()], outs=[h_full[:, :].opt()])

            # ---- L2 scatter (node-major) + relu + pool ----
            ppool = ps_pool.tile([128, n_graphs], F32)
            ch = 0
            for t in range(ntile):
                pt = ps_agg.tile([128, oh], F32, tag="aggps")
                for j in range(int(cpt[t])):
                    k, cc = ch // CB, ch % CB
                    if cc == 0:
                        gi_t = mp.tile([128, CB], I32, tag="gi")
                        sd_t = mp.tile([128, CB], F32, tag="sd")
                        sw_t = mp.tile([128, CB], F32, tag="sw")
                        nc.sync.dma_start(out=gi_t[:, :], in_=t_gi[k, :, :])
                        nc.sync.dma_start(out=sd_t[:, :], in_=t_sd[k, :, :])
                        nc.sync.dma_start(out=sw_t[:, :], in_=t_sw[k, :, :])
                    g_t = gp.tile([128, oh], F32, tag="g")
                    nc.gpsimd.indirect_dma_start(
                        out=g_t[:, :], out_offset=None, in_=h_full[:, :],
                        in_offset=bass.IndirectOffsetOnAxis(
                            ap=gi_t[:, cc : cc + 1], axis=0))
                    s_t = sp.tile([128, 128], F32, tag="s")
                    nc.vector.tensor_scalar(
                        out=s_t[:, :], in0=iota[:, :],
                        scalar1=sd_t[:, cc : cc + 1], scalar2=sw_t[:, cc : cc + 1],
                        op0=mybir.AluOpType.is_equal, op1=mybir.AluOpType.mult)
                    nc.tensor.matmul(pt[:, :], lhsT=s_t[:, :], rhs=g_t[:, :],
                                     start=(j == 0), stop=(j == int(cpt[t]) - 1))
                    ch += 1
                h2 = stp.tile([128, oh], F32, tag="h2")
                nc.vector.tensor_tensor(out=h2[:, :], in0=pt[:, :], in1=b2r[:, :],
                                        op=mybir.AluOpType.add)
                nc.vector.tensor_scalar(
                    out=h2[:, :], in0=h2[:, :], scalar1=0.0, scalar2=None,
                    op0=mybir.AluOpType.max)
                pm_t = sp.tile([128, n_graphs], F32, tag="pm")
                nc.vector.tensor_scalar(
                    out=pm_t[:, :], in0=iota[:, :n_graphs],
                    scalar1=pms[:, 2 * t : 2 * t + 1],
                    scalar2=pms[:, 2 * t + 1 : 2 * t + 2],
                    op0=mybir.AluOpType.is_equal, op1=mybir.AluOpType.mult)
                nc.tensor.matmul(ppool[:, :], lhsT=h2[:, :], rhs=pm_t[:, :],
                                 start=(t == 0), stop=(t == ntile - 1))

            # ---- AllReduce pooled, FC ----
            ar_in = ccp.tile([128, n_graphs], F32)
            ar_out = ccp.tile([128, n_graphs], F32, addr_space="Shared")
            pooled = stp.tile([128, n_graphs], F32, tag="pooled")
            nc.vector.tensor_copy(pooled[:, :], ppool[:, :])
            nc.sync.dma_start(out=ar_in[:, :], in_=pooled[:, :])
            nc.gpsimd.collective_compute(
                "AllReduce", mybir.AluOpType.add,
                replica_groups=[list(range(NCORES))],
                ins=[ar_in[:, :].opt()], outs=[ar_out[:, :].opt()])
            pfull = stp.tile([128, n_graphs], F32, tag="pfull")
            nc.sync.dma_start(out=pfull[:, :], in_=ar_out[:, :])
            pfc = ps_fc.tile([n_graphs, 8], F32)
            nc.tensor.matmul(pfc[:, :], lhsT=pfull[:, :], rhs=wfc[:, :],
                             start=True, stop=True)
            osb = stp.tile([n_graphs, 8], F32, tag="osb")
            nc.vector.tensor_tensor(out=osb[:, :], in0=pfc[:, :], in1=bfc[:, :],
                                    op=mybir.AluOpType.add)
            nc.sync.dma_start(out=t_out[:, :], in_=osb[:, :])
    nc.compile()
    return nc


def kernel(x, src, dst, batch, W1, b1, W2, b2, Wfc, bfc):
    global last_result
    x = np.asarray(x, np.float32)
    src = np.asarray(src, np.int64)
    dst = np.asarray(dst, np.int64)
    batch = np.asarray(batch, np.int64)
    W1, b1v, W2, b2v, Wfc, bfcv = (np.asarray(a, np.float32)
                                   for a in (W1, b1, W2, b2, Wfc, bfc))
    n, in_dim = x.shape
    hid = W1.shape[1]
    oh = W2.shape[1]
    ng = 64
    odim = Wfc.shape[1]

    meta = _plan(src, dst, n)
    npc, ntile, ncalls = meta["npc"], meta["ntile"], meta["ncalls"]

    key = (n, in_dim, hid, oh, tuple(meta["cpt"]))
    if key not in _cache:
        _cache[key] = _build(meta, n, in_dim, hid, oh, ng)
    nc = _cache[key]

    iota = np.tile(np.arange(128, dtype=np.float32), (128, 1))
    eye = np.eye(128, dtype=np.float32)
    cnt = np.maximum(np.bincount(batch, minlength=ng).astype(np.float32), 1.0)
    b2r = np.tile(b2v.reshape(1, oh), (128, 1)).astype(np.float32)
    wfc8 = np.zeros((oh, 8), np.float32)
    wfc8[:, :odim] = Wfc
    bfc8 = np.zeros((ng, 8), np.float32)
    bfc8[:, :odim] = bfcv.reshape(1, odim)
    b1t = np.ascontiguousarray(b1v.reshape(hid // 128, 128).T)

    ins = []
    for c in range(NCORES):
        gs, sd, sw = meta["cores"][c]
        bslot = np.zeros(ntile * 128, np.float32)
        binv = np.zeros(ntile * 128, np.float32)
        nl = np.arange(npc) + c * npc
        bslot[:npc] = batch[nl].astype(np.float32)
        binv[:npc] = 1.0 / cnt[batch[nl]]
        pms = np.zeros((128, ntile * 2), np.float32)
        pms[:, 0::2] = bslot.reshape(ntile, 128).T
        pms[:, 1::2] = binv.reshape(ntile, 128).T
        ins.append({
            "xs": np.ascontiguousarray(x[c * npc : (c + 1) * npc]),
            "gi": _pack_calls(gs, ncalls),
            "sd": _pack_calls(sd, ncalls),
            "sw": _pack_calls(sw, ncalls),
            "w1": W1, "b1": b1t, "w2": W2, "b2r": b2r,
            "pms": pms,
            "wfc": wfc8, "bfc": bfc8, "iota": iota, "eye": eye,
        })
    import time as _t
    _s = _t.time()
    r = run_bass_kernel_spmd(nc, ins, core_ids=list(range(NCORES)))
    exec_wall[0] = _t.time() - _s
    last_result = (r,)
    return np.asarray(r.results[0]["out"][:, :odim], np.float32)
